# revision 1
# baseline (speedup 1.0000x reference)
"""Trainium2 Bass kernel: causal multi-head self-attention (B=4, T=4096, D=128, H=4, dh=32).

Sharding: 8 cores = 4 batches x 2 head-pairs. Core c handles batch c//2, heads
{2*(c%2), 2*(c%2)+1}. Each core returns a partial output projection (its 2 heads'
contribution); the host sums the two partials per batch.

Algorithm per (head, q-super of 512 queries):
  scores^T[j, q] = K^T-block (zero-padded K=128 lhsT) @ Q^T  -> PSUM
  E = exp(scores^T * 1/sqrt(dh))  (fp32r out, no max-subtraction: scores are O(3))
  E *= causal mask on diagonal blocks
  O^T[d|l, q] += [V | ones | 0]^T-block @ E   (accumulated over j-blocks in PSUM;
                                               row 32 = softmax denominator l)
  Y_h[q, o] = (O^T slice)^T @ W_out-slice ; out = Y_h0/l0 + Y_h1/l1 (per-partition scale)
"""

import math
import numpy as np

import concourse.bass as bass
import concourse.bacc as bacc
import concourse.mybir as mybir
import concourse.tile as tile
from concourse import bass_utils

F32 = mybir.dt.float32
F32R = mybir.dt.float32r
Exp = mybir.ActivationFunctionType.Exp

B, T, D = 4, 4096, 128
H, DH = 4, 32
NCORES = 8
NQS = T // 512          # 8 q-supers
NJB = T // 128          # 32 j-blocks
GROUP = 3               # j-blocks per S-psum group (3 banks per slot, double buffered)
SCALE = 1.0 / math.sqrt(DH)

# matmul dtypes (switchable): S-stage (Q/K) and O-stage (V/E)
BF16 = mybir.dt.bfloat16
DT_S = F32R
DT_O = F32R


def _round_f32r(x: np.ndarray) -> np.ndarray:
    """Round-to-nearest-even fp32 -> fp32r (11 explicit mantissa bits, low 12 dropped)."""
    u = np.ascontiguousarray(x, dtype=np.float32).view(np.uint32)
    half = np.uint32(1 << 11)
    lsb = (u >> np.uint32(12)) & np.uint32(1)
    u = ((u + half - np.uint32(1) + lsb) >> np.uint32(12)) << np.uint32(12)
    return u.view(np.float32)


def _to_dt(x: np.ndarray, dt) -> np.ndarray:
    if dt == F32R:
        return _round_f32r(x)
    if dt == mybir.dt.bfloat16:
        import ml_dtypes
        return np.ascontiguousarray(x, dtype=np.float32).astype(ml_dtypes.bfloat16)
    return np.ascontiguousarray(x, dtype=np.float32)


def build_program() -> bacc.Bacc:
    nc = bacc.Bacc("TRN2", target_bir_lowering=False, debug=False, num_devices=NCORES)

    # ---- DRAM I/O (per core) ----
    xt_d = nc.dram_tensor("xt", [D, T], DT_S, kind="ExternalInput").ap()
    wq_d = [nc.dram_tensor(f"wq{h}", [D, 128], DT_S, kind="ExternalInput").ap() for h in range(2)]
    wk_d = [nc.dram_tensor(f"wk{h}", [D, 128], DT_S, kind="ExternalInput").ap() for h in range(2)]
    wv_d = nc.dram_tensor("wv", [D, 64], DT_S, kind="ExternalInput").ap()
    wo_d = [nc.dram_tensor(f"wo{h}", [128, 128], DT_S, kind="ExternalInput").ap() for h in range(2)]
    mask_d = nc.dram_tensor("mask", [128, 896], DT_O, kind="ExternalInput").ap()
    y_d = nc.dram_tensor("y", [T, D], F32, kind="ExternalOutput").ap()
    l_d = nc.dram_tensor("ldram", [2, NQS, 512], F32, kind="Internal").ap()

    with tile.TileContext(nc) as tc:
        with (
            tc.tile_pool(name="const", bufs=1) as cpool,
            tc.tile_pool(name="epool", bufs=4) as epool,
            tc.tile_pool(name="work", bufs=3) as wpool,
            tc.tile_pool(name="psS", bufs=2, space="PSUM") as psS,
            tc.tile_pool(name="psO", bufs=2, space="PSUM") as psO,
        ):
            # ---- persistent SBUF ----
            xt = cpool.tile([D, T], DT_S)
            wq = [cpool.tile([D, 128], DT_S, name=f"wq{h}", tag=f"wq{h}") for h in range(2)]
            wk = [cpool.tile([D, 128], DT_S, name=f"wk{h}", tag=f"wk{h}") for h in range(2)]
            wv = cpool.tile([D, 64], DT_S)
            wo = [cpool.tile([128, 128], DT_S, name=f"wo{h}", tag=f"wo{h}") for h in range(2)]
            mask = cpool.tile([128, 896], DT_O)
            qt = [cpool.tile([128, T], DT_S, name=f"qt{h}", tag=f"qt{h}") for h in range(2)]
            ktz = [cpool.tile([128, T], DT_S, name=f"ktz{h}", tag=f"ktz{h}") for h in range(2)]
            vx = [cpool.tile([128, T], DT_O, name=f"vx{h}", tag=f"vx{h}") for h in range(2)]
            osb = [cpool.tile([128, T], DT_S, name=f"osb{h}", tag=f"osb{h}") for h in range(2)]
            lcol = [cpool.tile([128, NJB], F32, name=f"lcol{h}", tag=f"lcol{h}") for h in range(2)]
            rl = [cpool.tile([128, NJB], F32, name=f"rl{h}", tag=f"rl{h}") for h in range(2)]
            ytmp = cpool.tile([128, T], F32)

            # ---- weight / mask loads ----
            nc.sync.dma_start(xt[:, 0:512], xt_d[:, 0:512])
            for h in range(2):
                nc.sync.dma_start(wq[h][:, :], wq_d[h][:, :])
                nc.sync.dma_start(wk[h][:, :], wk_d[h][:, :])
            nc.sync.dma_start(wv[:, :], wv_d[:, :])
            nc.sync.dma_start(mask[:, :], mask_d[:, :])
            for h in range(2):
                nc.sync.dma_start(wo[h][:, :], wo_d[h][:, :])
            # vx pattern: zero everything, then ones in col 32 of each 128-wide j-block
            for h in range(2):
                if DT_O == F32R:
                    nc.gpsimd.memset(vx[h][:, :].bitcast(F32), 0.0)
                    for jb in range(NJB):
                        nc.gpsimd.memset(vx[h][:, 128 * jb + 32 : 128 * jb + 33].bitcast(F32), 1.0)
                else:
                    nc.gpsimd.memset(vx[h][:, :], 0.0)
                    for jb in range(NJB):
                        nc.gpsimd.memset(vx[h][:, 128 * jb + 32 : 128 * jb + 33], 1.0)

            def emit_qkv(qs):
                sl = slice(512 * qs, 512 * (qs + 1))
                if qs > 0:
                    nc.sync.dma_start(xt[:, sl], xt_d[:, sl])
                for dst, w in ((qt[0], wq[0]), (qt[1], wq[1]), (ktz[0], wk[0]), (ktz[1], wk[1])):
                    p = psO.tile([128, 512], F32, name="p", tag="po")
                    nc.tensor.matmul(p[:, :], w[:, :], xt[:, sl], start=True, stop=True)
                    nc.vector.tensor_copy(dst[:, sl], p[:, :])
                for jb in range(4 * qs, 4 * qs + 4):
                    jsl = slice(128 * jb, 128 * (jb + 1))
                    p = psO.tile([128, 512], F32, name="p", tag="po")
                    nc.tensor.matmul(p[:, 0:64], xt[:, jsl], wv[:, :], start=True, stop=True)
                    nc.vector.tensor_copy(vx[0][:, 128 * jb : 128 * jb + 32], p[:, 0:32])
                    nc.vector.tensor_copy(vx[1][:, 128 * jb : 128 * jb + 32], p[:, 32:64])

            def emit_attn(h, qs):
                qsl = slice(512 * qs, 512 * (qs + 1))
                njb = 4 * (qs + 1)
                o_ps = psO.tile([128, 512], F32, name="o_ps", tag="po")
                # descending j-blocks: diagonal (masked) groups first, so their DVE
                # mask ops overlap later groups instead of sitting on the qs-boundary
                # critical chain.
                order = list(range(njb - 1, -1, -1))
                groups = [order[i : i + GROUP] for i in range(0, njb, GROUP)]
                for gi, jbs in enumerate(groups):
                    n = len(jbs)
                    s_ps = psS.tile([128, 512 * GROUP], F32, name="s_ps", tag="s")
                    for k, jb in enumerate(jbs):
                        nc.tensor.matmul(
                            s_ps[:, 512 * k : 512 * (k + 1)],
                            ktz[h][:, 128 * jb : 128 * (jb + 1)],
                            qt[h][:, qsl],
                            start=True, stop=True,
                        )
                    e = epool.tile([128, 512 * GROUP], DT_O, name="e", tag="e")
                    nc.scalar.activation(e[:, 0 : 512 * n], s_ps[:, 0 : 512 * n], Exp, scale=SCALE)
                    for k, jb in enumerate(jbs):
                        g = jb - 4 * qs
                        if g >= 0:
                            esl = slice(512 * k, 512 * (k + 1))
                            nc.vector.tensor_mul(e[:, esl], e[:, esl], mask[:, 384 - 128 * g : 896 - 128 * g])
                    for k, jb in enumerate(jbs):
                        nc.tensor.matmul(
                            o_ps[:, :],
                            vx[h][:, 128 * jb : 128 * (jb + 1)],
                            e[:, 512 * k : 512 * (k + 1)],
                            start=(gi == 0 and k == 0), stop=(gi == len(groups) - 1 and k == n - 1),
                        )
                # evacuate: full 128 rows (rows 33..127 are zeros; row 32 = l)
                nc.vector.tensor_copy(osb[h][:, qsl], o_ps[:, :])
                lrow = wpool.tile([1, 512], F32, name="lrow", tag="lrow")
                nc.vector.tensor_copy(lrow[0:1, :], o_ps[32:33, :])
                # l -> per-partition column layout via DRAM bounce + reciprocal
                d1 = nc.sync.dma_start(l_d[h, qs, :], lrow[0:1, :])
                lsrc = l_d[h, qs, :].rearrange("(b c) -> c b", c=128)
                d2 = nc.sync.dma_start(lcol[h][:, 4 * qs : 4 * qs + 4], lsrc)
                tile.add_dep_helper(d2.ins, d1.ins, reason="l dram bounce RAW")
                nc.vector.reciprocal(rl[h][:, 4 * qs : 4 * qs + 4], lcol[h][:, 4 * qs : 4 * qs + 4])

            def emit_proj(h, qs):
                for qb in range(4 * qs, 4 * qs + 4):
                    bsl = slice(128 * qb, 128 * (qb + 1))
                    p = psO.tile([128, 512], F32, name="p", tag="po")
                    nc.tensor.matmul(p[:, 0:128], osb[h][:, bsl], wo[h][:, :], start=True, stop=True)
                    if h == 0:
                        nc.vector.tensor_scalar_mul(ytmp[:, bsl], p[:, 0:128], rl[0][:, qb : qb + 1])
                    else:
                        ty = wpool.tile([128, 128], F32, name="ty", tag="ty")
                        nc.vector.scalar_tensor_tensor(
                            ty[:, :], p[:, 0:128], rl[1][:, qb : qb + 1], ytmp[:, bsl],
                            op0=mybir.AluOpType.mult, op1=mybir.AluOpType.add,
                        )
                        nc.sync.dma_start(y_d[bsl, :], ty[:, :])

            # ---- qkv interleaved with attention; heads interleaved per q-super ----
            with nc.named_scope("attn"):
                for qs in range(NQS):
                    emit_qkv(qs)
                    emit_attn(0, qs)
                    emit_attn(1, qs)
            # ---- phase D: projections (emitted last; scheduler fills gaps) ----
            with nc.named_scope("proj"):
                for qs in range(NQS):
                    emit_proj(0, qs)
                    emit_proj(1, qs)

    nc.compile()
    return nc


def make_in_maps(x: np.ndarray, W_qkv: np.ndarray, W_out: np.ndarray):
    """Host-side shard prep: per-core input dict."""
    x = np.asarray(x, dtype=np.float32)
    W_qkv = np.asarray(W_qkv, dtype=np.float32)
    W_out = np.asarray(W_out, dtype=np.float32)

    # sliding causal mask master: master[jp, c] = 1.0 if jp <= c - 384
    # diagonal-block g uses master[:, 384-128g : 896-128g] == (128g + jp <= ql)
    jp = np.arange(128)[:, None]
    cc = np.arange(896)[None, :]
    mask = (jp <= cc - 384).astype(np.float32)
    mask = _to_dt(mask, DT_O)

    in_maps = []
    for c in range(NCORES):
        b = c // 2
        h0 = 2 * (c % 2)
        xt = _to_dt(x[b].T, DT_S)
        m = {"xt": xt, "mask": mask}
        for i, h in enumerate((h0, h0 + 1)):
            wq_pad = np.zeros((D, 128), np.float32)
            wq_pad[:, 0:32] = W_qkv[32 * h : 32 * (h + 1), :].T
            wk_pad = np.zeros((D, 128), np.float32)
            wk_pad[:, 0:32] = W_qkv[128 + 32 * h : 128 + 32 * (h + 1), :].T
            wo_pad = np.zeros((128, 128), np.float32)
            wo_pad[0:32, :] = W_out[:, 32 * h : 32 * (h + 1)].T
            m[f"wq{i}"] = _to_dt(wq_pad, DT_S)
            m[f"wk{i}"] = _to_dt(wk_pad, DT_S)
            m[f"wo{i}"] = _to_dt(wo_pad, DT_S)
        m["wv"] = _to_dt(W_qkv[256 + 32 * h0 : 256 + 32 * h0 + 64, :].T, DT_S)
        in_maps.append(m)
    return in_maps


_PROGRAM_CACHE = {}


def kernel(x: np.ndarray, W_qkv: np.ndarray, W_out: np.ndarray, _trace=False, _tmpdir=None) -> np.ndarray:
    if "nc" not in _PROGRAM_CACHE:
        _PROGRAM_CACHE["nc"] = build_program()
    nc = _PROGRAM_CACHE["nc"]

    in_maps = make_in_maps(x, W_qkv, W_out)
    res = bass_utils.run_bass_kernel_spmd(
        nc, in_maps, core_ids=list(range(NCORES)), trace=_trace, tmpdir=_tmpdir
    )
    out = np.empty((B, T, D), np.float32)
    for b in range(B):
        out[b] = res.results[2 * b]["y"] + res.results[2 * b + 1]["y"]
    if _trace:
        kernel.last_result = res
    return out



# revision 2
# speedup vs baseline: 1.0561x; 1.0561x over previous
"""Trainium2 Bass kernel: causal multi-head self-attention (B=4, T=4096, D=128, H=4, dh=32).

Sharding: 8 cores = 4 batches x 2 head-pairs. Core c handles batch c//2, heads
{2*(c%2), 2*(c%2)+1}. Each core emits per-head unnormalized projections Y_h and
softmax denominators l_h; the host computes sum_h Y_h / l_h per batch.

Per (head, q-super of 512 queries), all matmuls in bf16/fp16 (1 cyc/col):
  S^T[j,q]  : 2x row-tiled K=32 matmuls (tile_position rows 0/32) — Q^T/K^T are
              stored replicated on two 32-partition strips.
  causal mask: for diagonal j-blocks, a constant matmul (identity @ Tg) adds
              -1e9 above the diagonal directly in PSUM — no vector mask work.
  E = exp   : head 0 of the pair on ScalarE (real exp); head 1 on VectorE via a
              custom DVE op relu(s*a+b) -> int16, whose bits read as bf16 give
              2^(s*log2e) (Schraudolph) — masked entries clamp to +0.0.
  O^T      += 2x col-tiled M=64 matmuls ([V_j | ones | pad] stationary); the two
              64-partition strips are summed for free in the output projection
              by replicating W_out along partitions; moving column 128 of the
              projection extracts the summed denominator l.
"""

import math
import numpy as np

import concourse.bass as bass
import concourse.bacc as bacc
import concourse.mybir as mybir
import concourse.tile as tile
from concourse import bass_utils
import concourse.dve_ops as dve_ops
from concourse.dve_spec import Spec, Src0, C0, C1, relu, lower
from concourse.dve_uop import DveOpSpec

F32 = mybir.dt.float32
BF16 = mybir.dt.bfloat16
FP16 = mybir.dt.float16
I16 = mybir.dt.int16
Exp = mybir.ActivationFunctionType.Exp

B, T, D = 4, 4096, 128
H, DH = 4, 32
NCORES = 8
NQS = T // 512
SCALE = 1.0 / math.sqrt(DH)
NEG = -1.0e9

# Schraudolph bf16-bit exp: bf16_bits(e^s) ~= round(s*log2e*128 + (127+sigma)*128)
SIGMA = -0.03
SCHR_A = math.log(2.0, 2.0)  # placeholder; real consts below
SCHR_A = (1.0 / math.log(2.0)) * SCALE * 128.0
SCHR_B = (127.0 + SIGMA) * 128.0


def _register_exp2():
    name = "EXP2_BITS_ANT"
    for op in dve_ops.OPS:
        if op.name == name:
            return op
    spec = Spec(body=relu(Src0 * C0 + C1))
    row = dve_ops._CUSTOM_DVE_ROW_BASE + len(dve_ops.OPS)
    assert row < 0x20
    shas = {}
    for ver in ("v3", "v4"):
        try:
            s = DveOpSpec(name=name, opcode=row, uops=lower(spec, ver=ver), rd1_en=False)
            shas[ver] = s.sha(ver)
        except Exception:
            pass
    dve_ops._SUB_OPCODE_FOR_NAME[name] = row
    op = dve_ops.DveOp(name, spec, subdim=False, uops_sha=shas)
    dve_ops.OPS.append(op)
    dve_ops.CUSTOM_DVE_SPECS[name] = spec
    return op


EXP2_OP = _register_exp2()


def build_program() -> bacc.Bacc:
    nc = bacc.Bacc("TRN2", target_bir_lowering=False, debug=False, num_devices=NCORES)

    xt_d = nc.dram_tensor("xt", [D, T], BF16, kind="ExternalInput").ap()
    wqk_d = [nc.dram_tensor(f"wqk{h}", [D, 128], BF16, kind="ExternalInput").ap() for h in range(2)]
    wv_d = nc.dram_tensor("wv", [D, 64], BF16, kind="ExternalInput").ap()
    wo_d = [nc.dram_tensor(f"wo{h}", [128, 132], FP16, kind="ExternalInput").ap() for h in range(2)]
    tmask_d = nc.dram_tensor("tmask", [128, 2048], BF16, kind="ExternalInput").ap()
    ident_d = nc.dram_tensor("ident", [128, 128], BF16, kind="ExternalInput").ap()
    y_d = nc.dram_tensor("y", [2, NQS * 4, 128, 132], FP16, kind="ExternalOutput").ap()

    with tile.TileContext(nc) as tc:
        with (
            tc.tile_pool(name="const", bufs=1) as cpool,
            tc.tile_pool(name="epool", bufs=4) as epool,
            tc.tile_pool(name="ypool", bufs=4) as ypool,
            tc.tile_pool(name="psS", bufs=3, space="PSUM") as psS,
            tc.tile_pool(name="psO", bufs=2, space="PSUM") as psO,
        ):
            # ---- persistent SBUF ----
            xt = cpool.tile([D, T], BF16)
            wqk = [cpool.tile([D, 128], BF16, name=f"wqk{h}", tag=f"wqk{h}") for h in range(2)]
            wv = cpool.tile([D, 64], BF16)
            wo = [cpool.tile([128, 132], FP16, name=f"wo{h}", tag=f"wo{h}") for h in range(2)]
            tmask = cpool.tile([128, 2048], BF16)
            ident = cpool.tile([128, 128], BF16)
            qt = [cpool.tile([64, T], BF16, name=f"qt{h}", tag=f"qt{h}") for h in range(2)]
            kt = [cpool.tile([64, T], BF16, name=f"kt{h}", tag=f"kt{h}") for h in range(2)]
            vx = [cpool.tile([128, 64 * 32], BF16, name=f"vx{h}", tag=f"vx{h}") for h in range(2)]
            osb = [cpool.tile([128, T], FP16, name=f"osb{h}", tag=f"osb{h}") for h in range(2)]

            # ---- init loads ----
            nc.sync.dma_start(xt[:, 0:512], xt_d[:, 0:512])
            for h in range(2):
                nc.sync.dma_start(wqk[h][:, :], wqk_d[h][:, :])
                nc.sync.dma_start(wo[h][:, :], wo_d[h][:, :])
            nc.sync.dma_start(wv[:, :], wv_d[:, :])
            nc.sync.dma_start(tmask[:, :], tmask_d[:, :])
            nc.sync.dma_start(ident[:, :], ident_d[:, :])
            for h in range(2):
                nc.gpsimd.memset(vx[h][:, :], 0.0)
                for jb in range(32):
                    nc.gpsimd.memset(vx[h][:, 64 * jb + 32 : 64 * jb + 33], 1.0)

            def copy_h(h, out, in_):
                """PSUM->SBUF evacuations: head 0 on ScalarE, head 1 on VectorE."""
                if h == 0:
                    nc.scalar.copy(out, in_)
                else:
                    nc.vector.tensor_copy(out, in_)

            def emit_qkv(qs):
                qsl = slice(512 * qs, 512 * (qs + 1))
                for h in range(2):
                    p = psS.tile([128, 1024], F32, name="p", tag="s")
                    nc.tensor.matmul(p[:, 0:512], wqk[h][:, :], xt[:, qsl], start=True, stop=True)
                    copy_h(h, qt[h][0:64, qsl], p[0:64, 0:512])
                    copy_h(h, kt[h][0:64, qsl], p[64:128, 0:512])
                pv = psS.tile([128, 1024], F32, name="pv", tag="s")
                for k in range(4):
                    jsl = slice(512 * qs + 128 * k, 512 * qs + 128 * (k + 1))
                    nc.tensor.matmul(pv[:, 64 * k : 64 * k + 64], xt[:, jsl], wv[:, :], start=True, stop=True)
                for h in range(2):
                    src = pv[:, 0:256].rearrange("p (n s) -> p n s", s=64)[:, :, 32 * h : 32 * h + 32]
                    dst = vx[h][:, 256 * qs * 1 : 256 * (qs + 1)].rearrange("p (n s) -> p n s", s=64)[:, :, 0:32]
                    copy_h(h, dst, src)
                if qs + 1 < NQS:
                    nsl = slice(512 * (qs + 1), 512 * (qs + 2))
                    nc.sync.dma_start(xt[:, nsl], xt_d[:, nsl])

            def emit_attn(qs):
                qsl = slice(512 * qs, 512 * (qs + 1))
                njb = 4 * (qs + 1)
                npairs = njb // 2
                o_ps = [psO.tile([128, 512], F32, name=f"o{h}", tag="o") for h in range(2)]
                s_tiles = {}
                e_tiles = {}

                def emit_S(h, gi):
                    s = psS.tile([128, 1024], F32, name="s", tag="s")
                    s_tiles[(h, gi)] = s
                    for k in range(2):
                        jb = 2 * gi + k
                        g = jb - 4 * qs
                        ssl = slice(512 * k, 512 * (k + 1))
                        nc.tensor.matmul(
                            s[:, ssl],
                            kt[h][32 * k : 32 * (k + 1), 128 * jb : 128 * (jb + 1)],
                            qt[h][32 * k : 32 * (k + 1), qsl],
                            start=True,
                            stop=(g < 0),
                        )
                        if g >= 0:
                            nc.tensor.matmul(
                                s[:, ssl],
                                ident[:, :],
                                tmask[:, 512 * g : 512 * (g + 1)],
                                start=False,
                                stop=True,
                            )

                def emit_E(h, gi):
                    s = s_tiles.pop((h, gi))
                    e = epool.tile([128, 1024], BF16, name="e", tag="e")
                    e_tiles[(h, gi)] = e
                    if h == 0:
                        nc.scalar.activation(e[:, :], s[:, :], Exp, scale=SCALE)
                    else:
                        nc.vector._custom_dve(
                            EXP2_OP, out=e[:, :].bitcast(I16), in0=s[:, :], s0=SCHR_A, s1=SCHR_B
                        )

                def emit_O(h, gi):
                    e = e_tiles.pop((h, gi))
                    for k in range(2):
                        jb = 2 * gi + k
                        nc.tensor.matmul(
                            o_ps[h][64 * k : 64 * (k + 1), :],
                            vx[h][:, 64 * jb : 64 * (jb + 1)],
                            e[:, 512 * k : 512 * (k + 1)],
                            start=(gi == 0),
                            stop=(gi == npairs - 1),
                        )

                emit_S(0, 0)
                emit_S(1, 0)
                for gi in range(npairs):
                    emit_E(0, gi)
                    emit_E(1, gi)
                    if gi + 1 < npairs:
                        emit_S(0, gi + 1)
                        emit_S(1, gi + 1)
                    emit_O(0, gi)
                    emit_O(1, gi)
                return o_ps

            def emit_osb(qs, o_ps):
                qsl = slice(512 * qs, 512 * (qs + 1))
                for h in range(2):
                    copy_h(h, osb[h][:, qsl], o_ps[h][:, :])

            def emit_proj(qs):
                for h in range(2):
                    p = psS.tile([128, 1024], F32, name="pp", tag="s")
                    for lqb in range(4):
                        qb = 4 * qs + lqb
                        nc.tensor.matmul(
                            p[:, 256 * lqb : 256 * lqb + 132],
                            osb[h][:, 128 * qb : 128 * (qb + 1)],
                            wo[h][:, :],
                            start=True,
                            stop=True,
                        )
                    yb = ypool.tile([128, 4, 132], FP16, name="yb", tag="y")
                    src = p[:, 0:1024].rearrange("p (n s) -> p n s", s=256)[:, :, 0:132]
                    copy_h(h, yb[:, :, :], src)
                    dst = y_d[h, 4 * qs : 4 * qs + 4].rearrange("n p c -> p n c")
                    nc.sync.dma_start(dst, yb[:, :, :])

            with nc.named_scope("attn"):
                o_prev = None
                for qs in range(NQS):
                    emit_qkv(qs)
                    if qs > 0:
                        emit_osb(qs - 1, o_prev)
                    o_cur = emit_attn(qs)
                    if qs > 0:
                        emit_proj(qs - 1)
                    o_prev = o_cur
                emit_osb(NQS - 1, o_prev)
                emit_proj(NQS - 1)

    nc.compile()
    return nc


def _to_bf16(x: np.ndarray) -> np.ndarray:
    import ml_dtypes

    return np.ascontiguousarray(x, dtype=np.float32).astype(ml_dtypes.bfloat16)


def make_in_maps(x: np.ndarray, W_qkv: np.ndarray, W_out: np.ndarray):
    x = np.asarray(x, dtype=np.float32)
    W_qkv = np.asarray(W_qkv, dtype=np.float32)
    W_out = np.asarray(W_out, dtype=np.float32)

    # tmask[p, 512g + l] = NEG where 128g + p > l (upper triangle of diag block g)
    p = np.arange(128)[:, None]
    l = np.arange(512)[None, :]
    tmask = np.zeros((128, 2048), np.float32)
    for g in range(4):
        tmask[:, 512 * g : 512 * (g + 1)] = np.where(128 * g + p > l, NEG, 0.0)
    tmask = _to_bf16(tmask)
    ident = _to_bf16(np.eye(128, dtype=np.float32))

    in_maps = []
    for c in range(NCORES):
        b = c // 2
        h0 = 2 * (c % 2)
        m = {
            "xt": _to_bf16(x[b].T),
            "tmask": tmask,
            "ident": ident,
        }
        for i in range(2):
            h = h0 + i
            wqk = np.zeros((D, 128), np.float32)
            for k in range(2):
                wqk[:, 32 * k : 32 * k + 32] = W_qkv[32 * h : 32 * h + 32, :].T
                wqk[:, 64 + 32 * k : 64 + 32 * k + 32] = W_qkv[128 + 32 * h : 128 + 32 * h + 32, :].T
            m[f"wqk{i}"] = _to_bf16(wqk)
            woi = np.zeros((128, 132), np.float32)
            woi[0:32, 0:128] = W_out[:, 32 * h : 32 * h + 32].T
            woi[64:96, 0:128] = W_out[:, 32 * h : 32 * h + 32].T
            woi[32, 128] = 1.0
            woi[96, 128] = 1.0
            m[f"wo{i}"] = woi.astype(np.float16)
        m["wv"] = _to_bf16(W_qkv[256 + 32 * h0 : 256 + 32 * h0 + 64, :].T)
        in_maps.append(m)
    return in_maps


_PROGRAM_CACHE = {}


def kernel(x: np.ndarray, W_qkv: np.ndarray, W_out: np.ndarray, _trace=False, _tmpdir=None) -> np.ndarray:
    if "nc" not in _PROGRAM_CACHE:
        _PROGRAM_CACHE["nc"] = build_program()
    nc = _PROGRAM_CACHE["nc"]

    in_maps = make_in_maps(x, W_qkv, W_out)
    res = bass_utils.run_bass_kernel_spmd(
        nc, in_maps, core_ids=list(range(NCORES)), trace=_trace, tmpdir=_tmpdir
    )
    out = np.zeros((B, T, D), np.float32)
    for c in range(NCORES):
        b = c // 2
        y = np.asarray(res.results[c]["y"], dtype=np.float32)  # [2, 32, 128, 132]
        for i in range(2):
            yi = y[i].reshape(T, 132)
            out[b] += yi[:, 0:128] / yi[:, 128:129]
    if _trace:
        kernel.last_result = res
    return out
